# revision 1
# baseline (speedup 1.0000x reference)
"""Trainium2 Bass kernel for nn_DagEncoder (segment_reduce).

Computes, for N nodes grouped into B contiguous segments by a CSR ptr:
    h   = relu(concat([x, h_node], 1) @ W1 + b1)        # [N, H]
    out = segment_sum(h @ W2 + b2, seg)                 # [B, E]

Key algebraic restructure: segment_sum is linear, so
    out[b] = (sum_{i in b} h1_i) @ W2 + cnt_b * b2
which moves the second matmul from N rows to B rows (~61x less work).

Per-core device program (SPMD, identical on all 8 cores):
  - nodes are streamed in 128-node chunks, feature-major (host pre-transposed,
    bf16): mm1 = lhsT(dataT chunk) @ W1 -> PSUM [nodes, H], relu -> SBUF fp16
  - segment-sum via one-hot selector matmul: Sel[i, j] = (segloc[i] == j),
    built on VectorE with is_equal(iota, segloc); matmul(lhsT=h1, rhs=Sel)
    accumulates into a PSUM "window" [H, 128 segs] across all chunks of the
    window
  - window epilogue: drain window, mm2 with W2 (fp32) + outer(cnt, b2),
    write [128, E] f32 to DRAM.

Host packs whole segments into fixed-size windows (CPW chunks x <=128 segs,
~1% padding) so the instruction stream is identical across cores; dummy pad
nodes have zero data and segloc=-5 (never matches the iota), contributing 0.
"""

import sys

sys.path.insert(0, "/opt/trn_rl_repo")

from contextlib import ExitStack

import numpy as np
import ml_dtypes

# ---------------------------------------------------------------- constants
N = 2_000_000
F = 16
E = 128
H = 128
B = 32_768
NCORES = 8
CHUNK = 128          # nodes per chunk (matmul M / K limit)
SEG_W = 64           # segment window width (Sel matmul N, PSUM window cols)
GRP = 8              # chunks per relu/Sel group
FXA = F + 1          # x features + constant-1 bias feature

bf16 = ml_dtypes.bfloat16


# ---------------------------------------------------------------- host plan
def _plan_core(seglen, s0, s1, cpw):
    """Greedy-pack segments [s0, s1) into windows of <= cpw*CHUNK node slots
    and <= SEG_W segments. Returns list of (seg_start, nsegs, nnodes)."""
    slots = cpw * CHUNK
    wins = []
    seg_start, nsegs, used = s0, 0, 0
    for s in range(s0, s1):
        ln = int(seglen[s])
        if nsegs > 0 and (used + ln > slots or nsegs >= SEG_W):
            wins.append((seg_start, nsegs, used))
            seg_start, nsegs, used = s, 0, 0
        assert ln <= slots, f"segment {s} len {ln} > window slots {slots}"
        nsegs += 1
        used += ln
    if nsegs > 0:
        wins.append((seg_start, nsegs, used))
    return wins


def _build_program(nw, cpw, dtd=None, dth=None, passes=1):
    """Build the SPMD Bass/Tile program for nw windows of cpw chunks.

    passes>1 repeats the whole body (same inputs/outputs) inside one launch —
    used only for device-time measurement via T(2 passes) - T(1 pass)."""
    import concourse.bacc as bacc
    import concourse.bass as bass
    import concourse.tile as tile
    from concourse import mybir

    if dtd is None:
        dtd = mybir.dt.bfloat16    # data / W1 dtype
    if dth is None:
        dth = mybir.dt.float16     # h1 / Sel dtype
    f32 = mybir.dt.float32
    Relu = mybir.ActivationFunctionType.Relu
    Copy = mybir.ActivationFunctionType.Copy
    slots = cpw * CHUNK
    assert cpw % GRP == 0

    nc = bacc.Bacc(None, target_bir_lowering=False, debug=False)

    hT = nc.dram_tensor("hT", [H, nw * slots], dtd, kind="ExternalInput")
    xT = nc.dram_tensor("xT", [FXA, nw * slots], dtd, kind="ExternalInput")
    segloc = nc.dram_tensor("segloc", [CHUNK, nw * cpw], dth, kind="ExternalInput")
    cnt = nc.dram_tensor("cnt", [1, nw * SEG_W], f32, kind="ExternalInput")
    w1h = nc.dram_tensor("w1h", [H, H], dtd, kind="ExternalInput")
    w1x = nc.dram_tensor("w1x", [FXA, H], dtd, kind="ExternalInput")
    w2 = nc.dram_tensor("w2", [H, E], f32, kind="ExternalInput")
    b2r = nc.dram_tensor("b2r", [1, E], f32, kind="ExternalInput")
    iota = nc.dram_tensor("iota", [CHUNK, GRP * SEG_W], dth, kind="ExternalInput")
    out = nc.dram_tensor("out", [nw * SEG_W, E], f32, kind="ExternalOutput")

    with tile.TileContext(nc) as tc, ExitStack() as ctx:
        consts = ctx.enter_context(tc.tile_pool(name="consts", bufs=1))
        data_p = ctx.enter_context(tc.tile_pool(name="data", bufs=2))
        segl_p = ctx.enter_context(tc.tile_pool(name="segl", bufs=2))
        h1_p = ctx.enter_context(tc.tile_pool(name="h1", bufs=3))
        sel_p = ctx.enter_context(tc.tile_pool(name="sel", bufs=3))
        win_p = ctx.enter_context(tc.tile_pool(name="win", bufs=2))
        out_p = ctx.enter_context(tc.tile_pool(name="outp", bufs=2))
        ps_mm1 = ctx.enter_context(tc.tile_pool(name="psmm1", bufs=2, space="PSUM"))
        ps_win = ctx.enter_context(tc.tile_pool(name="pswin", bufs=2, space="PSUM"))
        ps_out = ctx.enter_context(tc.tile_pool(name="psout", bufs=2, space="PSUM"))

        w1h_sb = consts.tile([H, H], dtd)
        nc.sync.dma_start(w1h_sb[:], w1h[:])
        w1x_sb = consts.tile([FXA, H], dtd)
        nc.sync.dma_start(w1x_sb[:], w1x[:])
        w2_sb = consts.tile([H, E], f32)
        nc.sync.dma_start(w2_sb[:], w2[:])
        b2_sb = consts.tile([1, E], f32)
        nc.sync.dma_start(b2_sb[:], b2r[:])
        iota_sb = consts.tile([CHUNK, GRP * SEG_W], dth)
        nc.sync.dma_start(iota_sb[:], iota[:])
        cnt_sb = consts.tile([1, nw * SEG_W], f32)
        nc.sync.dma_start(cnt_sb[:], cnt[:])

        gcols = GRP * CHUNK
        for w in range(nw * passes):
            w = w % nw
            win_ps = ps_win.tile([H, SEG_W], f32)
            segl_sb = segl_p.tile([CHUNK, cpw], dth)
            nc.sync.dma_start(segl_sb[:], segloc[:, w * cpw:(w + 1) * cpw])
            hT_sb = data_p.tile([H, slots], dtd, tag="hT")
            nc.sync.dma_start(hT_sb[:], hT[:, w * slots:(w + 1) * slots])
            xT_sb = data_p.tile([FXA, slots], dtd, tag="xT")
            nc.sync.dma_start(xT_sb[:], xT[:, w * slots:(w + 1) * slots])
            for g in range(cpw // GRP):
                g0 = g * gcols
                mm1_ps = ps_mm1.tile([CHUNK, gcols], f32)
                for j in range(GRP):
                    sl = slice(g0 + j * CHUNK, g0 + (j + 1) * CHUNK)
                    psl = slice(j * CHUNK, (j + 1) * CHUNK)
                    nc.tensor.matmul(mm1_ps[:, psl], hT_sb[:, sl], w1h_sb[:],
                                     start=True, stop=False)
                    nc.tensor.matmul(mm1_ps[:, psl], xT_sb[:, sl], w1x_sb[:],
                                     start=False, stop=True)
                h1_sb = h1_p.tile([CHUNK, gcols], dth)
                nc.scalar.activation(h1_sb[:], mm1_ps[:], Relu)

                # Sel for all GRP chunks in one DVE op: broadcast each chunk's
                # per-node seg id over SEG_W columns against a tiled iota
                sel_sb = sel_p.tile([CHUNK, GRP * SEG_W], dth)
                segl_b = segl_sb[:, g * GRP:(g + 1) * GRP].broadcast_to(
                    (CHUNK, GRP, SEG_W))
                nc.vector.tensor_tensor(
                    sel_sb[:].rearrange("p (j k) -> p j k", j=GRP),
                    iota_sb[:].rearrange("p (j k) -> p j k", j=GRP),
                    segl_b, mybir.AluOpType.is_equal)
                for j in range(GRP):
                    c = g * GRP + j
                    nc.tensor.matmul(win_ps[:],
                                     h1_sb[:, j * CHUNK:(j + 1) * CHUNK],
                                     sel_sb[:, j * SEG_W:(j + 1) * SEG_W],
                                     start=(c == 0), stop=(c == cpw - 1))

            # window epilogue: [H, SEG_W] seg-sums of h1 -> @W2 + cnt*b2
            win_sb = win_p.tile([H, SEG_W], f32)
            nc.scalar.activation(win_sb[:], win_ps[:], Copy)
            out_ps = ps_out.tile([SEG_W, E], f32)
            nc.tensor.matmul(out_ps[:], win_sb[:], w2_sb[:],
                             start=True, stop=False)
            nc.tensor.matmul(out_ps[:], cnt_sb[:, w * SEG_W:(w + 1) * SEG_W],
                             b2_sb[:], start=False, stop=True)
            out_sb = out_p.tile([SEG_W, E], f32)
            nc.scalar.activation(out_sb[:], out_ps[:], Copy)
            nc.sync.dma_start(out[w * SEG_W:(w + 1) * SEG_W, :], out_sb[:])

    nc.compile()
    return nc


# ------------------------------------------------------------- host packing
def _pack_core(x, h_node, seg_of_node, seglen, s0, s1, n0, n1, wins, nw, cpw):
    """Build one core's padded input arrays."""
    slots = cpw * CHUNK
    tot = nw * slots
    nn = n1 - n0

    # global node index where each window's real nodes begin
    wnode0 = np.empty(len(wins), np.int64)
    run = n0
    for i, (_, _, nnod) in enumerate(wins):
        wnode0[i] = run
        run += nnod
    g = np.arange(n0, n1)
    wid = np.searchsorted(wnode0, g, side="right") - 1
    slot = wid * slots + (g - wnode0[wid])

    hT = np.zeros((H, tot), bf16)
    hT[:, slot] = h_node[n0:n1].T.astype(bf16)
    xT = np.zeros((FXA, tot), bf16)
    xT[:F, slot] = x[n0:n1].T.astype(bf16)
    xT[F, slot] = bf16(1.0)

    segf = np.full(tot, -5.0, np.float16)
    wseg0 = np.array([wv[0] for wv in wins], np.int64)
    segf[slot] = (seg_of_node[g] - wseg0[wid]).astype(np.float16)
    segloc = np.ascontiguousarray(segf.reshape(nw * cpw, CHUNK).T)

    cnt = np.zeros((1, nw * SEG_W), np.float32)
    for i, (ss, nsg, _) in enumerate(wins):
        cnt[0, i * SEG_W:i * SEG_W + nsg] = seglen[ss:ss + nsg]
    return {"hT": hT, "xT": xT, "segloc": segloc, "cnt": cnt}


_PROG_CACHE = {}
LAST_CTX = None   # (nc, in_maps, plans, nw, cpw) of the most recent run


def kernel(x, h_node, ptr, W1, b1, W2, b2):
    x = np.asarray(x, np.float32)
    h_node = np.asarray(h_node, np.float32)
    ptr = np.asarray(ptr, np.int64)
    W1 = np.asarray(W1, np.float32)
    b1 = np.asarray(b1, np.float32)
    W2 = np.asarray(W2, np.float32)
    b2 = np.asarray(b2, np.float32)

    seglen = np.diff(ptr)
    seg_of_node = np.repeat(np.arange(B, dtype=np.int64), seglen)

    spc = B // NCORES
    cpw = 32
    while seglen.max() > cpw * CHUNK:
        cpw += GRP
    plans = []
    for k in range(NCORES):
        s0, s1 = k * spc, (k + 1) * spc
        plans.append(_plan_core(seglen, s0, s1, cpw))
    nw = max(len(p) for p in plans)

    key = (nw, cpw)
    if key not in _PROG_CACHE:
        _PROG_CACHE[key] = _build_program(nw, cpw)
    nc = _PROG_CACHE[key]

    # shared constant inputs
    w1x_aug = np.zeros((FXA, H), np.float32)
    w1x_aug[:F] = W1[:F]
    w1x_aug[F] = b1
    const_maps = {
        "w1h": W1[F:].astype(bf16),
        "w1x": w1x_aug.astype(bf16),
        "w2": W2.astype(np.float32),
        "b2r": b2.reshape(1, E).astype(np.float32),
        "iota": np.broadcast_to(
            np.tile(np.arange(SEG_W, dtype=np.float16), GRP),
            (CHUNK, GRP * SEG_W)).copy(),
    }

    in_maps = []
    for k in range(NCORES):
        s0, s1 = k * spc, (k + 1) * spc
        n0, n1 = int(ptr[s0]), int(ptr[s1])
        m = _pack_core(x, h_node, seg_of_node, seglen, s0, s1, n0, n1,
                       plans[k], nw, cpw)
        m.update(const_maps)
        in_maps.append(m)

    global LAST_CTX
    LAST_CTX = (nc, in_maps, plans, nw, cpw)

    from concourse.bass_utils import run_bass_kernel_spmd

    res = run_bass_kernel_spmd(nc, in_maps, list(range(NCORES)))

    out = np.zeros((B, E), np.float32)
    for k in range(NCORES):
        o = res.results[k]["out"]
        for i, (ss, nsg, _) in enumerate(plans[k]):
            out[ss:ss + nsg] = o[i * SEG_W:i * SEG_W + nsg]
    return out



# revision 6
# speedup vs baseline: 4.8622x; 4.8622x over previous
"""Trainium2 Bass kernel for nn_DagEncoder (segment_reduce), v2.

Computes, for N nodes grouped into B contiguous segments by a CSR ptr:
    h   = relu(concat([x, h_node], 1) @ W1 + b1)        # [N, H]
    out = segment_sum(h @ W2 + b2, seg)                 # [B, E]

Algebra: segment_sum is linear, so
    out[b] = (sum_{i in b} h1_i) @ W2 + cnt_b * b2
moving the second matmul from N rows to ~B rows.

v2 design (vs v1's per-chunk weight-reload scheme):
  - mm1 runs "W-stationary": W1 is the PE stationary operand, node columns
    stream as the moving operand in 512-col blocks -> out z.T [H, nodes] in
    PSUM. 2 streamed cols/node total (h-part K=128 + x-part K=17), no
    per-chunk LDWEIGHTS of data.
  - relu drains PSUM->SBUF fp16, split between ScalarE (ACT) and VectorE
    (tensor_scalar_max) by a tunable ratio.
  - segment reduction: host splits segments into pieces of <= T=64 nodes,
    sorts pieces by length, packs groups of S=128 similar-length pieces as a
    position-major grid (col = pos*S + lane). The per-group reduce is then a
    pairwise tree of contiguous tensor_tensor adds on the DVE (fp16, 2x
    mode). ~1.5% padding.
  - mm2 per group: win [H, S] (tree output) as lhsT, stream W2 -> out
    [S, E] psum, + cnt x b2 rank-1 matmul for the bias, ACT copy, DMA out.
  - host re-accumulates piece rows into segment rows (np.add.at).
"""

import sys

sys.path.insert(0, "/opt/trn_rl_repo")

from contextlib import ExitStack

import numpy as np
import ml_dtypes

# ---------------------------------------------------------------- constants
N = 2_000_000
F = 16
E = 128
H = 128
B = 32_768
NCORES = 8
S = 128            # pieces (lanes) per group = mm2 output partitions
T = 64             # max piece length (positions per group)
NB = 512           # matmul moving-block columns (one PSUM bank, fp32)
PSB = 1536         # PSUM tile columns (3 banks) = relu drain granularity
FXA = F + 1        # x features + constant-1 bias feature
DVE_RELU_EVERY = 5 # every k-th PSUM tile relu goes to VectorE instead of ACT

bf16 = ml_dtypes.bfloat16


# ---------------------------------------------------------------- host plan
def _plan(seglen):
    """Split segments into pieces of <= T nodes, sort by length, group into
    groups of 8*S pieces (S lanes x 8 cores). Returns piece arrays (rank
    order), per-group position counts Lg, and group col bases."""
    npc = np.maximum(1, np.ceil(seglen / T).astype(np.int64))
    P = int(npc.sum())
    pseg = np.repeat(np.arange(B, dtype=np.int64), npc)
    pidx = np.arange(P, dtype=np.int64) - np.repeat(np.cumsum(npc) - npc, npc)
    pstart = pidx * T
    plen = np.maximum(np.minimum(seglen[pseg] - pstart, T), 0)

    order = np.argsort(-plen, kind="stable")
    pseg, pstart, plen = pseg[order], pstart[order], plen[order]

    gsz = NCORES * S
    Ppad = int(np.ceil(P / gsz) * gsz)
    pad = Ppad - P
    pseg = np.concatenate([pseg, np.full(pad, -1, np.int64)])
    pstart = np.concatenate([pstart, np.zeros(pad, np.int64)])
    plen = np.concatenate([plen, np.zeros(pad, np.int64)])

    G = Ppad // gsz
    Lg = np.maximum(plen[::gsz][:G], 1)
    base = np.concatenate([[0], np.cumsum(S * Lg)]).astype(np.int64)
    return P, Ppad, pseg, pstart, plen, G, Lg, base


# ---------------------------------------------------------------- program
def _build_program(plan_key, passes=1):
    """Build the SPMD Bass/Tile program. plan_key = (G, tuple(Lg)).

    passes>1 repeats the body (same buffers) for device-time measurement via
    T(2 passes) - T(1 pass)."""
    import concourse.bacc as bacc
    import concourse.bass as bass
    import concourse.tile as tile
    from concourse import mybir

    G, Lg = plan_key
    Lg = list(Lg)
    d_in = mybir.dt.bfloat16
    d_h1 = mybir.dt.float16
    f32 = mybir.dt.float32
    Relu = mybir.ActivationFunctionType.Relu
    Copy = mybir.ActivationFunctionType.Copy
    Add = mybir.AluOpType.add

    base = [0]
    for L in Lg:
        base.append(base[-1] + S * L)
    TOT = base[-1]

    nc = bacc.Bacc(None, target_bir_lowering=False, debug=False)

    hT = nc.dram_tensor("hT", [H, TOT], d_in, kind="ExternalInput")
    xaT = nc.dram_tensor("xaT", [FXA, TOT], d_in, kind="ExternalInput")
    cnt = nc.dram_tensor("cnt", [1, G * S], f32, kind="ExternalInput")
    w1h = nc.dram_tensor("w1h", [H, H], d_in, kind="ExternalInput")
    w1xa = nc.dram_tensor("w1xa", [FXA, H], d_in, kind="ExternalInput")
    w2 = nc.dram_tensor("w2", [H, E], d_h1, kind="ExternalInput")
    b2r = nc.dram_tensor("b2r", [1, E], f32, kind="ExternalInput")
    out = nc.dram_tensor("out", [G * S, E], f32, kind="ExternalOutput")

    with tile.TileContext(nc) as tc, ExitStack() as ctx:
        consts = ctx.enter_context(tc.tile_pool(name="consts", bufs=1))
        data_p = ctx.enter_context(tc.tile_pool(name="data", bufs=3))
        xa_p = ctx.enter_context(tc.tile_pool(name="xa", bufs=3))
        h1_p = ctx.enter_context(tc.tile_pool(name="h1", bufs=2))
        s1_p = ctx.enter_context(tc.tile_pool(name="s1", bufs=2))
        s2_p = ctx.enter_context(tc.tile_pool(name="s2", bufs=2))
        out_p = ctx.enter_context(tc.tile_pool(name="outp", bufs=2))
        ps_p = ctx.enter_context(tc.tile_pool(name="ps", bufs=2, space="PSUM"))
        ps_out = ctx.enter_context(tc.tile_pool(name="psout", bufs=2,
                                                space="PSUM"))

        w1h_sb = consts.tile([H, H], d_in)
        nc.sync.dma_start(w1h_sb[:], w1h[:])
        w1xa_sb = consts.tile([FXA, H], d_in)
        nc.sync.dma_start(w1xa_sb[:], w1xa[:])
        w2_sb = consts.tile([H, E], d_h1)
        nc.sync.dma_start(w2_sb[:], w2[:])
        b2_sb = consts.tile([1, E], f32)
        nc.sync.dma_start(b2_sb[:], b2r[:])
        cnt_sb = consts.tile([1, G * S], f32)
        nc.sync.dma_start(cnt_sb[:], cnt[:])

        tile_ctr = 0
        for g in range(G * passes):
            g = g % G
            L = Lg[g]
            cols = S * L
            b0 = base[g]

            hT_sb = data_p.tile([H, cols], d_in, tag="hT")
            nc.sync.dma_start(hT_sb[:], hT[:, b0:b0 + cols])
            xa_sb = xa_p.tile([FXA, cols], d_in, tag="xaT")
            nc.sync.dma_start(xa_sb[:], xaT[:, b0:b0 + cols])

            h1_sb = h1_p.tile([H, cols], d_h1, tag="h1")

            # mm1 + relu in PSUM tiles of up to PSB columns
            for t0 in range(0, cols, PSB):
                tcols = min(PSB, cols - t0)
                ps = ps_p.tile([H, tcols], f32)
                for c0 in range(0, tcols, NB):
                    nb = min(NB, tcols - c0)
                    nc.tensor.matmul(ps[:, c0:c0 + nb], w1h_sb[:],
                                     hT_sb[:, t0 + c0:t0 + c0 + nb],
                                     start=True, stop=False)
                for c0 in range(0, tcols, NB):
                    nb = min(NB, tcols - c0)
                    nc.tensor.matmul(ps[:, c0:c0 + nb], w1xa_sb[:],
                                     xa_sb[:, t0 + c0:t0 + c0 + nb],
                                     start=False, stop=True)
                dst = h1_sb[:, t0:t0 + tcols]
                if tile_ctr % DVE_RELU_EVERY == DVE_RELU_EVERY - 1:
                    nc.vector.tensor_scalar_max(dst, ps[:], 0.0)
                else:
                    nc.scalar.activation(dst, ps[:], Relu)
                tile_ctr += 1

            # pairwise tree over positions: cur[i] + cur[rest+i] -> nxt[i]
            cur, curL = h1_sb, L
            lvl = 0
            while curL > 1:
                half = curL // 2
                rest = curL - half
                pool = s1_p if lvl % 2 == 0 else s2_p
                nxt = pool.tile([H, rest * S], d_h1)
                nc.vector.tensor_tensor(nxt[:, 0:half * S],
                                        cur[:, 0:half * S],
                                        cur[:, rest * S:curL * S], Add)
                if rest > half:  # odd: carry the middle row
                    nc.vector.tensor_copy(nxt[:, half * S:rest * S],
                                          cur[:, half * S:rest * S])
                cur, curL = nxt, rest
                lvl += 1
            win = cur[:, 0:S]  # [H, S] fp16 segment (piece) sums

            # mm2: out[s, e] = win.T @ W2 + cnt*b2
            out_ps = ps_out.tile([S, E], f32)
            nc.tensor.matmul(out_ps[:], win, w2_sb[:], start=True, stop=False)
            nc.tensor.matmul(out_ps[:], cnt_sb[:, g * S:(g + 1) * S],
                             b2_sb[:], start=False, stop=True)
            out_sb = out_p.tile([S, E], f32)
            nc.scalar.activation(out_sb[:], out_ps[:], Copy)
            nc.sync.dma_start(out[g * S:(g + 1) * S, :], out_sb[:])

    nc.compile()
    return nc


# ------------------------------------------------------------- host packing
def _pack_core(k, x, h_node, ptr, Ppad, pseg, pstart, plen, G, base, TOT):
    rk = np.arange(k, Ppad, NCORES)
    lens = plen[rk]
    segs = pseg[rk]
    starts = pstart[rk]
    nk = int(lens.sum())

    cum = np.cumsum(lens) - lens
    p = np.arange(nk, dtype=np.int64) - np.repeat(cum, lens)
    ii = np.repeat(np.arange(len(rk), dtype=np.int64), lens)
    g = ii // S
    j = ii % S
    col = base[g] + p * S + j
    node = np.repeat(ptr[np.maximum(segs, 0)] + starts, lens) + p

    hbuf = np.zeros((TOT, H), bf16)
    hbuf[col] = h_node[node].astype(bf16)
    xbuf = np.zeros((TOT, FXA), bf16)
    xbuf[col, :F] = x[node].astype(bf16)
    xbuf[col, F] = bf16(1.0)

    cnt = lens.astype(np.float32).reshape(1, G * S)
    return {"hT": np.ascontiguousarray(hbuf.T),
            "xaT": np.ascontiguousarray(xbuf.T),
            "cnt": cnt}


_PROG_CACHE = {}
LAST_CTX = None   # (nc, in_maps, plan_key) of the most recent run


def kernel(x, h_node, ptr, W1, b1, W2, b2):
    x = np.asarray(x, np.float32)
    h_node = np.asarray(h_node, np.float32)
    ptr = np.asarray(ptr, np.int64)
    W1 = np.asarray(W1, np.float32)
    b1 = np.asarray(b1, np.float32)
    W2 = np.asarray(W2, np.float32)
    b2 = np.asarray(b2, np.float32)

    seglen = np.diff(ptr)
    P, Ppad, pseg, pstart, plen, G, Lg, base = _plan(seglen)
    TOT = int(base[-1])

    plan_key = (G, tuple(int(v) for v in Lg))
    if plan_key not in _PROG_CACHE:
        _PROG_CACHE[plan_key] = _build_program(plan_key)
    nc = _PROG_CACHE[plan_key]

    w1xa = np.zeros((FXA, H), np.float32)
    w1xa[:F] = W1[:F]
    w1xa[F] = b1
    const_maps = {
        "w1h": W1[F:].astype(bf16),
        "w1xa": w1xa.astype(bf16),
        "w2": W2.astype(np.float16),
        "b2r": b2.reshape(1, E).astype(np.float32),
    }

    in_maps = []
    for k in range(NCORES):
        m = _pack_core(k, x, h_node, ptr, Ppad, pseg, pstart, plen, G,
                       base, TOT)
        m.update(const_maps)
        in_maps.append(m)

    global LAST_CTX
    LAST_CTX = (nc, in_maps, plan_key)

    from concourse.bass_utils import run_bass_kernel_spmd

    res = run_bass_kernel_spmd(nc, in_maps, list(range(NCORES)))

    # re-accumulate piece rows into segment rows
    rows = np.empty((Ppad, E), np.float32)
    for k in range(NCORES):
        rows[np.arange(k, Ppad, NCORES)] = res.results[k]["out"]
    out_full = np.zeros((B, E), np.float32)
    np.add.at(out_full, pseg[:P], rows[:P])
    return out_full
